# revision 5
# baseline (speedup 1.0000x reference)
"""Trainium2 kernel for nn_DoubleAffineNet.

Math: the module's output is phi + psi - I where phi, psi are 3x3 affine
matrices built from pooled image statistics. phi needs mean(x), mean(y).
psi needs mean(x) and mean(y_comp), where y_comp is y bilinearly warped by
the near-identity affine map phi^{-1}.

Key identity: only the MEAN of y_comp is needed. Writing the warp-mean as
sum_{p,q} Y[p,q] * G[p,q] (G = bilinear splat weights of the affinely
mapped output lattice), a partition-of-unity argument shows that for
sub-pixel displacement fields (|u|,|v| < 0.5, which holds for this
problem's near-identity maps; asserted at runtime on the host), G is the
constant kappa = (1-a')(1-d') + b*c everywhere except the four border
rows/cols. Hence

    sum(y_comp) = kappa * sum(y) + sum_border Y*(G_true - kappa)

The device kernel therefore only computes the memory-bound statistics:
per-sample sum(x), sum(y), and the four border strips of y. The remaining
O(B*(3x3 + 4*1024)) algebra runs on the host (f32 where the reference is
f32, f64 for the border correction).

Sharding: pure data parallel, one sample per NeuronCore (B=8, 8 cores).

Device program (raw bacc, no TileContext). The stream is DMA-bound
(~360-410 GB/s effective per core, varying with neighbor-core HBM
contention); the schedule minimizes the head and the tail around it:
  - 13 input DMAs on the sync HWDGE ring, in this order:
      YT        y[896:1024]           [128,1024]  (0.5 MB)
      X0..X3    x row blocks          [128,2048]  (1 MB each, rows 2p+a)
      Y0, Y1    y[0:256], y[256:512]  [128,2048]  (1 MB each)
      Y2a, Y2b  y[512:640], [640:768] [128,1024]  (0.5 MB each)
      tc1..tc4  y[768:896] col quarters [128,256] (0.125 MB each),
                strip-bearing quarters (cols 0-255, 768-1023) first
    All patterns are >=1 KB contiguous per partition. The row-1023 strip
    lives in YT which lands FIRST, so its strip DMA runs early; the last
    0.5 MB lands as four small chunks reduced back-to-back on vector (the
    strip-bearing quarters stream first so the late strips DMA also
    leaves before the stream ends), and the final reduce trails the
    stream end by only ~0.75 us.
  - Vector reduces X0-3, Y1, tc1-4; Scalar ACT-accumulates YT, Y0, Y2a,
    Y2b; GpSimd copies the border-column strips. Each chunk has its own
    semaphore (cumulative counts on one sem can release early under
    queue imbalance).
  - Outputs: row strips DMAed as soon as their chunk lands; partials and
    strips that are ready before the stream ends go out in an EARLY
    [128,20] DMA; the tail ships as a [128,5] partials DMA on the sync
    ring and a [128,4] strips DMA on the scalar HWDGE ring so their
    drains and HBM-write receipts overlap.
  - host does the final ~KB of reduction/algebra in float64
"""

import numpy as np

H = 1024
W = 1024
NE = 20   # early cols
NP = 5    # late partial cols (Y2b, tc1..tc4)
NS = 4    # late strip cols (Y2b c0, Y2b c1, tc1 c0, tc4 c1)
OFF_LP = 128 * NE                 # 2560
OFF_LS = OFF_LP + 128 * NP        # 3200
OFF_ROW0 = OFF_LS + 128 * NS      # 3712
OFF_R1 = OFF_ROW0 + W             # 4736
OUT_LEN = OFF_R1 + W              # 5760

_CACHE = {}


def _build_program():
    import contextlib

    import concourse.bacc as bacc
    from concourse import mybir

    f32 = mybir.dt.float32
    Copy = mybir.ActivationFunctionType.Copy
    nc = bacc.Bacc("TRN2", target_bir_lowering=False, debug=False, num_devices=8)

    xd = nc.dram_tensor("x", [H, W], f32, kind="ExternalInput").ap()
    yd = nc.dram_tensor("y", [H, W], f32, kind="ExternalInput").ap()
    outd = nc.dram_tensor("out", [OUT_LEN], f32, kind="ExternalOutput").ap()

    # 0 YT y[896:1024] [128,1024]; 1-4 X0..X3 [128,2048] rows 2p+a;
    # 5 Y0 y[0:256] 6 Y1 y[256:512] [128,2048]; 7 Y2a y[512:640] 8 Y2b
    # y[640:768] [128,1024]; 9-12 tc1..tc4 y[768:896] col quarters [128,256]
    free_cols = [1024, 2048, 2048, 2048, 2048, 2048, 2048, 1024, 1024,
                 256, 256, 256, 256]

    def src_chunk(k):
        if k == 0:
            return yd[896:1024, :]
        if k < 5:
            c = k - 1
            return xd[c * 256:(c + 1) * 256, :].rearrange("(p a) q -> p (a q)", a=2)
        if k == 5:
            return yd[0:256, :].rearrange("(p a) q -> p (a q)", a=2)
        if k == 6:
            return yd[256:512, :].rearrange("(p a) q -> p (a q)", a=2)
        if k == 7:
            return yd[512:640, :]
        if k == 8:
            return yd[640:768, :]
        # tail quarters ordered so the strip-bearing ones (cols 0-255 and
        # 768-1023) land first: the late strips DMA issues before the
        # stream ends, and the reduce pipeline starts one arrival earlier
        c = [0, 3, 1, 2][k - 9]
        return yd[768:896, c * 256:(c + 1) * 256]

    with contextlib.ExitStack() as ctx:
        bufs = [
            ctx.enter_context(nc.sbuf_tensor(f"chunk{k}", [128, free_cols[k]], f32))
            for k in range(13)
        ]
        # smalls columns:
        #  0-3 X0..X3  4 YT  5 Y0  6 Y1  7 Y2a   (partials, early)
        #  8,9 Y0 c0  10,11 Y1 c0  12 Y2a c0  13 YT c0
        #  14,15 Y0 c1  16,17 Y1 c1  18 Y2a c1  19 YT c1
        #  20 Y2b  21-24 tc1..tc4   (partials, late)
        #  25 Y2b c0  26 Y2b c1  27 tc1 c0  28 tc4 c1  (strips, late)
        smalls = ctx.enter_context(nc.sbuf_tensor("smalls", [128, NE + NP + NS], f32))
        scratch = ctx.enter_context(nc.sbuf_tensor("scratch", [128, 2048], f32))
        in_sem = [ctx.enter_context(nc.semaphore(f"in{k}")) for k in range(13)]
        done_e = ctx.enter_context(nc.semaphore("done_e"))
        done_red = ctx.enter_context(nc.semaphore("done_red"))
        done_str = ctx.enter_context(nc.semaphore("done_str"))
        dma_out = ctx.enter_context(nc.semaphore("dma_out"))
        block = ctx.enter_context(nc.Block(no_gpsimd_drain=True))

        @block.sync
        def _(sync):
            for k in range(13):
                sync.dma_start(out=bufs[k][:], in_=src_chunk(k)).then_inc(
                    in_sem[k], 16
                )
            # row strips: issued as soon as their chunk lands (early in the
            # stream by construction), so they never sit on the tail
            sync.wait_ge(in_sem[0], 16)
            sync.dma_start(
                out=outd[OFF_R1:OFF_R1 + W].rearrange("(p q) -> p q", p=1),
                in_=bufs[0][127:128, :],
            ).then_inc(dma_out, 16)
            sync.wait_ge(in_sem[5], 16)
            sync.dma_start(
                out=outd[OFF_ROW0:OFF_ROW0 + W].rearrange("(p q) -> p q", p=1),
                in_=bufs[5][0:1, 0:W],
            ).then_inc(dma_out, 16)
            sync.wait_ge(done_e, 12)
            sync.dma_start(
                out=outd[0:128 * NE].rearrange("(p c) -> p c", c=NE),
                in_=smalls[:, 0:NE],
            ).then_inc(dma_out, 16)
            sync.wait_ge(done_red, 5)
            sync.dma_start(
                out=outd[OFF_LP:OFF_LP + 128 * NP].rearrange("(p c) -> p c", c=NP),
                in_=smalls[:, NE:NE + NP],
            ).then_inc(dma_out, 16)
            sync.wait_ge(dma_out, 80)

        @block.vector
        def _(vector):
            for k, col, sem in [
                (1, 0, done_e),
                (2, 1, done_e),
                (3, 2, done_e),
                (4, 3, done_e),
                (6, 6, done_e),
                (9, 21, done_red),
                (10, 22, done_red),
                (11, 23, done_red),
                (12, 24, done_red),
            ]:
                vector.wait_ge(in_sem[k], 16)
                nc.vector.tensor_reduce(
                    out=smalls[:, col:col + 1],
                    in_=bufs[k][:],
                    axis=mybir.AxisListType.X,
                    op=mybir.AluOpType.add,
                ).then_inc(sem, 1)

        @block.scalar
        def _(scalar):
            for k, col, sem in [
                (0, 4, done_e),
                (5, 5, done_e),
                (7, 7, done_e),
                (8, 20, done_red),
            ]:
                scalar.wait_ge(in_sem[k], 16)
                nc.scalar.activation(
                    scratch[:, 0:free_cols[k]], bufs[k][:], Copy,
                    accum_out=smalls[:, col:col + 1],
                ).then_inc(sem, 1)
            # strips go out on the scalar HWDGE ring, overlapping the
            # partials DMA on the sync ring
            scalar.wait_ge(done_str, 3)
            scalar.dma_start(
                out=outd[OFF_LS:OFF_LS + 128 * NS].rearrange("(p c) -> p c", c=NS),
                in_=smalls[:, NE + NP:NE + NP + NS],
            ).then_inc(dma_out, 16)

        @block.gpsimd
        def _(gpsimd):
            # border-column strips; one done inc per chunk
            gpsimd.wait_ge(in_sem[0], 16)
            nc.gpsimd.tensor_copy(smalls[:, 13:14], bufs[0][:, 0:1])
            nc.gpsimd.tensor_copy(
                smalls[:, 19:20], bufs[0][:, 1023:1024]
            ).then_inc(done_e, 1)
            for k, c0, c1 in [(5, 8, 14), (6, 10, 16)]:
                gpsimd.wait_ge(in_sem[k], 16)
                t3 = bufs[k].ap().rearrange("p (a q) -> p a q", a=2)
                nc.gpsimd.tensor_copy(smalls[:, c0:c0 + 2], t3[:, :, 0])
                nc.gpsimd.tensor_copy(
                    smalls[:, c1:c1 + 2], t3[:, :, W - 1]
                ).then_inc(done_e, 1)
            gpsimd.wait_ge(in_sem[7], 16)
            nc.gpsimd.tensor_copy(smalls[:, 12:13], bufs[7][:, 0:1])
            nc.gpsimd.tensor_copy(
                smalls[:, 18:19], bufs[7][:, 1023:1024]
            ).then_inc(done_e, 1)
            gpsimd.wait_ge(in_sem[8], 16)
            nc.gpsimd.tensor_copy(smalls[:, 25:26], bufs[8][:, 0:1])
            nc.gpsimd.tensor_copy(
                smalls[:, 26:27], bufs[8][:, 1023:1024]
            ).then_inc(done_str, 1)
            gpsimd.wait_ge(in_sem[9], 16)
            nc.gpsimd.tensor_copy(
                smalls[:, 27:28], bufs[9][:, 0:1]
            ).then_inc(done_str, 1)
            gpsimd.wait_ge(in_sem[10], 16)
            nc.gpsimd.tensor_copy(
                smalls[:, 28:29], bufs[10][:, 255:256]
            ).then_inc(done_str, 1)

    nc.compile()
    return nc


def _get_program():
    if "nc" not in _CACHE:
        _CACHE["nc"] = _build_program()
    return _CACHE["nc"]


def _tent(z):
    return np.maximum(0.0, 1.0 - np.abs(z))


def _warp_mean_exact(y_img, A):
    """Fallback: honest bilinear warp-mean in numpy (used only if the
    sub-pixel displacement assumption fails, which it does not for this
    problem's inputs)."""
    A64 = A.astype(np.float64)
    i = np.arange(H, dtype=np.float64)[:, None]
    j = np.arange(W, dtype=np.float64)[None, :]
    px = A64[0, 0] * i + A64[0, 1] * j + 1023.0 * A64[0, 2]
    py = A64[1, 0] * i + A64[1, 1] * j + 1023.0 * A64[1, 2]
    x0 = np.floor(px).astype(np.int64)
    y0 = np.floor(py).astype(np.int64)
    wx = px - x0
    wy = py - y0
    im = y_img.astype(np.float64)
    acc = np.zeros((H, W))
    for xi, yi, w in (
        (x0, y0, (1 - wx) * (1 - wy)),
        (x0, y0 + 1, (1 - wx) * wy),
        (x0 + 1, y0, wx * (1 - wy)),
        (x0 + 1, y0 + 1, wx * wy),
    ):
        valid = (xi >= 0) & (xi < H) & (yi >= 0) & (yi < W)
        acc += im[np.clip(xi, 0, H - 1), np.clip(yi, 0, W - 1)] * w * valid
    return acc.mean()


def _warp_sum(sum_y, row0, row1, c0, c1, A):
    """sum(y_comp) from sum(y) + border strips, given phi_inv = A (f32).

    Requires the sub-pixel displacement assumption |u|,|v| < 0.5 (checked
    at the field corners; the fields are affine so corners bound the
    interior). The caller falls back to _warp_mean_exact otherwise.
    """
    A64 = A.astype(np.float64)
    ap, bb = A64[0, 0] - 1.0, A64[0, 1]
    cc, dp = A64[1, 0], A64[1, 1] - 1.0
    e1, e2 = 1023.0 * A64[0, 2], 1023.0 * A64[1, 2]

    mu = max(abs(ap * i + bb * j + e1) for i in (0.0, 1023.0) for j in (0.0, 1023.0))
    mv = max(abs(cc * i + dp * j + e2) for i in (0.0, 1023.0) for j in (0.0, 1023.0))
    assert mu < 0.5 and mv < 0.5, (mu, mv)

    kappa = (1.0 - ap) * (1.0 - dp) + bb * cc

    def g_true(p, q):
        g = np.zeros(np.broadcast(p, q).shape)
        for di in (-1, 0, 1):
            for dj in (-1, 0, 1):
                i_, j_ = p - di, q - dj
                valid = (i_ >= 0) & (i_ < H) & (j_ >= 0) & (j_ < W)
                z1 = ap * i_ + bb * j_ + e1 - di
                z2 = cc * i_ + dp * j_ + e2 - dj
                g += _tent(z1) * _tent(z2) * valid
        return g

    qs = np.arange(W, dtype=np.float64)
    ps = np.arange(1, H - 1, dtype=np.float64)
    ds = 0.0
    ds += np.sum(row0.astype(np.float64) * (g_true(0.0, qs) - kappa))
    ds += np.sum(row1.astype(np.float64) * (g_true(1023.0, qs) - kappa))
    ds += np.sum(c0[1:-1].astype(np.float64) * (g_true(ps, 0.0) - kappa))
    ds += np.sum(c1[1:-1].astype(np.float64) * (g_true(ps, 1023.0) - kappa))

    return kappa * float(sum_y) + ds


def _affine_f32(feat32, Wl, bl):
    M = (feat32 @ Wl + bl).reshape(3, 3)
    return np.eye(3, dtype=np.float32) + np.float32(0.01) * M


def _interleave2(a, b):
    return np.stack([a, b], axis=1).ravel()


def _unpack(r):
    """(sum_x, sum_y, row0, row1, c0, c1) from the flat out tensor."""
    sm_e = r[0:128 * NE].reshape(128, NE).astype(np.float64)
    sm_p = r[OFF_LP:OFF_LP + 128 * NP].reshape(128, NP).astype(np.float64)
    sm_s = r[OFF_LS:OFF_LS + 128 * NS].reshape(128, NS).astype(np.float64)
    sum_x = float(sm_e[:, 0:4].sum())
    sum_y = float(sm_e[:, 4:8].sum() + sm_p.sum())
    c0 = np.concatenate([
        _interleave2(sm_e[:, 8], sm_e[:, 9]),     # rows   0-255 (Y0)
        _interleave2(sm_e[:, 10], sm_e[:, 11]),   # rows 256-511 (Y1)
        sm_e[:, 12],                              # rows 512-639 (Y2a)
        sm_s[:, 0],                               # rows 640-767 (Y2b)
        sm_s[:, 2],                               # rows 768-895 (tc1)
        sm_e[:, 13],                              # rows 896-1023 (YT)
    ])
    c1 = np.concatenate([
        _interleave2(sm_e[:, 14], sm_e[:, 15]),
        _interleave2(sm_e[:, 16], sm_e[:, 17]),
        sm_e[:, 18],
        sm_s[:, 1],
        sm_s[:, 3],                               # rows 768-895 (tc4)
        sm_e[:, 19],
    ])
    row0 = r[OFF_ROW0:OFF_ROW0 + W].astype(np.float64)
    row1 = r[OFF_R1:OFF_R1 + W].astype(np.float64)
    return sum_x, sum_y, row0, row1, c0, c1


def kernel(x, y, Wpsi, bpsi, Wphi, bphi):
    from concourse import bass_utils

    B = x.shape[0]
    assert x.shape == (B, 1, H, W) and y.shape == (B, 1, H, W)

    nc = _get_program()
    in_maps = [
        {"x": np.ascontiguousarray(x[b, 0]), "y": np.ascontiguousarray(y[b, 0])}
        for b in range(B)
    ]
    results = bass_utils.run_bass_kernel_spmd(
        nc, in_maps, core_ids=list(range(B))
    ).results

    out = np.empty((B, 3, 3), dtype=np.float32)
    inv_hw = 1.0 / float(H * W)
    for b in range(B):
        r = np.asarray(results[b]["out"], dtype=np.float32).reshape(-1)
        sum_x, sum_y, row0, row1, c0, c1 = _unpack(r)
        mean_x = np.float32(sum_x * inv_hw)
        mean_y = np.float32(sum_y * inv_hw)
        phi = _affine_f32(np.array([mean_x, mean_y], np.float32), Wpsi, bpsi)
        A = np.linalg.inv(phi)
        try:
            mean_yc = np.float32(_warp_sum(sum_y, row0, row1, c0, c1, A) * inv_hw)
        except AssertionError:
            mean_yc = np.float32(_warp_mean_exact(y[b, 0], A))
        psi = _affine_f32(np.array([mean_x, mean_yc], np.float32), Wphi, bphi)
        out[b] = phi + psi - np.eye(3, dtype=np.float32)
    return out
